# revision 36
# baseline (speedup 1.0000x reference)
"""Trainium2 Bass kernel for nn_ExpertPool (moe_routing).

Strategy (expert-parallel over 8 cores):
  - Token b only needs its own expert's MLP output, so instead of the dense
    8x dispatch we gather tokens by expert on the host and give expert e's
    tokens to core e (counts ~1024 each, padded to CAP).
  - All activations stay in [feature, token] layout on device, so every
    layer is out = lhsT.T @ rhs with lhsT = weights; no transposes anywhere.
  - setup_inputs uses g=1, be=0, b=0 for every LayerNorm/bias parameter.
    LayerNorm's per-token 1/std commutes through ReLU (positive scale) and
    through the next matmul as a column scale, and both LayerNorm and the
    final l2norm are invariant to per-token positive scales -- so the
    variance/rsqrt path cancels exactly.  Mean subtraction is linear, so it
    folds into the weights host-side (W' = W - rowmean over output dim).
    The device kernel is just: relu(W1'.T x), relu(W2'.T h1), y = W3.T h2,
    feats = y/||y||, sim = (A.T h2)/||y|| with A = (W3 @ anchors_n.T)/TEMP.
  - Class-sim is computed only for the ~125 classes owned by the expert
    (all other logits are exactly -inf by the reference mask).
  - All weights ship in ONE packed [128, 3008] tensor / one DMA; x ships as
    one DMA per token block, prefetched upfront; feats+sim leave as one
    fused DMA each per block (DMA dispatch on the sync sequencer costs
    ~0.6us each, so few+large transfers matter).
"""

import numpy as np

E, B, D, H, C = 8, 8192, 512, 256, 1000
TEMP, EPS = 0.1, 1e-5
CAP = 1152          # per-expert token capacity (seed-0 max: 1040 axon / 1082 cpu)
CAPC = 160          # per-expert class capacity (seed-0 max: 134 axon / 140 cpu)
BLOCKS = [(0, 384), (384, 384), (768, 384)]
MC = [(0, 128), (128, 32)]  # class-chunk (start, size) covering CAPC

# packed weight tensors: wpa = W1, wp2 = ones+W2, wp3 = W3+A
_W1_O = 0              # 4 chunks x 256
_WPA_COLS = _W1_O + 4 * H
_ONES_O = 0
_W2_O = 128            # 2 chunks x 256
_WP2_COLS = _W2_O + 2 * H
_W3_O = 0              # 2 chunks x 512
_A_O = _W3_O + 2 * D   # 2 chunks x 160
_WP3_COLS = _A_O + 2 * CAPC

_cache: dict = {}


def _patch_tile_drain():
    """Walrus in this env rejects >2 sync waits on the tail Drain (CTRL
    encoding limit). Split the waits into standalone wait instructions."""
    import concourse.tile as tile_mod
    from concourse.tile import ScopedClock
    from bass_rust import SemaphoreHandle

    if getattr(tile_mod.TileContext, "_drain_patched", False):
        return

    def _drain_and_barrier(self, tick_clock, wait_clock):
        nc = self.nc
        d1 = nc.sync.drain()
        wait_clock.add_sem_waits(
            d1.ins, ScopedClock({None: tick_clock.global_clock})
        )
        si = d1.ins.sync_info
        waits = list(si.on_wait) if si is not None else []
        if len(waits) > 2:
            d1.ins.sync_info = None
            for w in waits:
                nc.sync.wait_ge(SemaphoreHandle(w.ant_name, w.id), w.wait_value)
            nc.sync.drain()
        nc.all_engine_barrier(sem_only=True)
        assert self.sems is not None
        popped = nc._tile_sem_poison_stack.pop()
        assert popped is self._sem_poison
        nc.clear_and_free_semaphores(list(self.sems.allocated().values()))

    tile_mod.TileContext._drain_and_barrier = _drain_and_barrier
    tile_mod.TileContext._drain_patched = True


def _build_program():
    from contextlib import ExitStack

    import concourse.bacc as bacc
    import concourse.mybir as mybir
    from concourse.tile import TileContext, add_dep_helper

    _patch_tile_drain()

    f32 = mybir.dt.float32
    f32r = mybir.dt.float32r
    AF = mybir.ActivationFunctionType

    nc = bacc.Bacc("TRN2", target_bir_lowering=False, debug=False, num_devices=8)

    xp = nc.dram_tensor("xp", [128, 4, CAP], f32r, kind="ExternalInput")
    wpa = nc.dram_tensor("wpa", [128, _WPA_COLS], f32r, kind="ExternalInput")
    wp2 = nc.dram_tensor("wp2", [128, _WP2_COLS], f32r, kind="ExternalInput")
    wp3 = nc.dram_tensor("wp3", [128, _WP3_COLS], f32r, kind="ExternalInput")
    ft = nc.dram_tensor("ft", [128, 4, CAP], f32, kind="ExternalOutput")
    st = nc.dram_tensor("st", [128, 2, CAP], f32, kind="ExternalOutput")

    with TileContext(nc) as tc:
        with ExitStack() as ctx:
            wpool = ctx.enter_context(tc.tile_pool(name="wpool", bufs=1))
            xpool = ctx.enter_context(tc.tile_pool(name="xpool", bufs=1))
            hpool = ctx.enter_context(tc.tile_pool(name="hpool", bufs=2))
            qpool = ctx.enter_context(tc.tile_pool(name="qpool", bufs=2))
            opool = ctx.enter_context(tc.tile_pool(name="opool", bufs=2))
            psA = ctx.enter_context(tc.tile_pool(name="psA", bufs=2, space="PSUM"))
            psY = ctx.enter_context(tc.tile_pool(name="psY", bufs=4, space="PSUM"))
            psS = ctx.enter_context(tc.tile_pool(name="psS", bufs=2, space="PSUM"))

            wpa_sb = wpool.tile([128, _WPA_COLS], f32r, tag="wpa")
            d_wpa = nc.scalar.dma_start(out=wpa_sb, in_=wpa[:, :])
            wp2_sb = wpool.tile([128, _WP2_COLS], f32r, tag="wp2")
            d_wp2 = nc.scalar.dma_start(out=wp2_sb, in_=wp2[:, :])
            wp3_sb = wpool.tile([128, _WP3_COLS], f32r, tag="wp3")
            d_wp3 = nc.scalar.dma_start(out=wp3_sb, in_=wp3[:, :])
            ones_r = wp2_sb[:, _ONES_O:_ONES_O + 128]
            w1_sb = [wpa_sb[:, _W1_O + k * H:_W1_O + (k + 1) * H]
                     for k in range(4)]
            w2_sb = [wp2_sb[:, _W2_O + k * H:_W2_O + (k + 1) * H]
                     for k in range(2)]
            w3_sb = [wp3_sb[:, _W3_O + k * D:_W3_O + (k + 1) * D]
                     for k in range(2)]
            a_sb = [wp3_sb[:, _A_O + k * CAPC:_A_O + (k + 1) * CAPC]
                    for k in range(2)]
            eps_sb = wpool.tile([128, 1], f32, tag="eps")
            nc.vector.memset(eps_sb, 1e-30)
            scr_sb = wpool.tile([128, 1], f32, tag="scr")
            nc.scalar.activation(out=scr_sb, in_=eps_sb, func=AF.Sqrt)

            # PE warm-up: keep the HAM activity window busy while input DMAs
            # are in flight so real matmuls start at 2.4 GHz
            bf16 = mybir.dt.bfloat16
            warm = wpool.tile([128, 384], bf16, tag="warm")
            nc.vector.memset(warm, 0.0)
            wps = psS.tile([128, 256], f32, tag="sc")
            for i in range(20):
                nc.tensor.matmul(wps, lhsT=warm[:, 0:128],
                                 rhs=warm[:, 128:384],
                                 start=(i == 0), stop=(i == 19))

            # prefetch every token block upfront (one DMA per block)
            # x0 from the (otherwise idle) scalar queue so it overlaps wpa;
            # x1/x2 from sync, chained so they don't steal bandwidth from the
            # critical wpa+x0 transfers. The chain waits run on the sync
            # sequencer, which has nothing else to do until block-0 outputs.
            # x blocks all on the scalar HWDGE queue: its FIFO order gives
            # x0 -> x1 -> x2 while weights stream on the sync queue in
            # parallel (each engine has one HW queue; ~290 GB/s each)
            xts = []
            for bi, (off, NB) in enumerate(BLOCKS):
                t = xpool.tile([128, 4, NB], f32r, tag=f"xt{bi}")
                d = nc.sync.dma_start(out=t, in_=xp[:, :, off:off + NB])
                xts.append(t)

            for bi, (off, NB) in enumerate(BLOCKS):
                xt = xts[bi]

                # -------- layer 1: h1 = relu(W1'.T x) --------
                h1 = []
                for hc in range(2):
                    ps = psA.tile([128, NB], f32, tag="zps")
                    for k in range(4):
                        nc.tensor.matmul(
                            ps,
                            lhsT=w1_sb[k][:, hc * 128:(hc + 1) * 128],
                            rhs=xt[:, k, :],
                            start=(k == 0), stop=(k == 3),
                        )
                    h = hpool.tile([128, NB], f32r, tag=f"h1_{hc}")
                    nc.scalar.activation(out=h, in_=ps, func=AF.Relu)
                    h1.append(h)

                # -------- layer 2: h2 = relu(W2'.T h1) --------
                h2 = []
                for hc in range(2):
                    ps = psA.tile([128, NB], f32, tag="zps")
                    for k in range(2):
                        nc.tensor.matmul(
                            ps,
                            lhsT=w2_sb[k][:, hc * 128:(hc + 1) * 128],
                            rhs=h1[k][:, :],
                            start=(k == 0), stop=(k == 1),
                        )
                    h = hpool.tile([128, NB], f32r, tag=f"h2_{hc}")
                    nc.scalar.activation(out=h, in_=ps, func=AF.Relu)
                    h2.append(h)

                # -------- layer 3: y = W3.T h2 (stays in PSUM) --------
                y_ps = []
                for dc in range(4):
                    ps = psY.tile([128, NB], f32, tag="yps")
                    for k in range(2):
                        nc.tensor.matmul(
                            ps,
                            lhsT=w3_sb[k][:, dc * 128:(dc + 1) * 128],
                            rhs=h2[k][:, :],
                            start=(k == 0), stop=(k == 1),
                        )
                    y_ps.append(ps)

                # -------- ||y||^2 across partitions via ones-matmul --------
                ysq = []
                for dc in range(4):
                    q = qpool.tile([128, NB], f32r, tag=f"ysq{dc}")
                    nc.scalar.activation(out=q, in_=y_ps[dc], func=AF.Square)
                    ysq.append(q)
                ss_ps = psS.tile([128, NB], f32, tag="sc")
                for dc in range(4):
                    nc.tensor.matmul(
                        ss_ps, lhsT=ones_r, rhs=ysq[dc][:, :],
                        start=(dc == 0), stop=(dc == 3),
                    )
                std = qpool.tile([128, NB], f32, tag="std")
                nc.scalar.activation(out=std, in_=ss_ps, func=AF.Sqrt, bias=eps_sb)
                s_b = qpool.tile([128, NB], f32, tag="s_b")
                nc.vector.reciprocal_approx_fast(out=s_b, in_=std)

                # -------- feats = y * s --------
                last = (bi == len(BLOCKS) - 1)
                fo = opool.tile([128, 4, NB], f32, tag="fo")
                for dc in range(4):
                    nc.vector.tensor_mul(fo[:, dc, :], y_ps[dc], s_b)
                    if dc % 2 == 1:
                        nc.sync.dma_start(
                            out=ft[:, dc - 1:dc + 1, off:off + NB],
                            in_=fo[:, dc - 1:dc + 1, :])

                # -------- sim = (A.T h2) * s (A has 1/TEMP baked in) --------
                so = opool.tile([128, 2, NB], f32, tag="so")
                for mi, (mstart, msize) in enumerate(MC):
                    ps = psS.tile([128, NB], f32, tag="sc")
                    for k in range(2):
                        nc.tensor.matmul(
                            ps[0:msize, :],
                            lhsT=a_sb[k][:, mstart:mstart + msize],
                            rhs=h2[k][:, :],
                            start=(k == 0), stop=(k == 1),
                        )
                    nc.vector.tensor_mul(
                        so[0:msize, mi, :], ps[0:msize, :], s_b[0:msize, :]
                    )
                    if last:
                        nc.scalar.dma_start(
                            out=st[:, mi:mi + 1, off:off + NB],
                            in_=so[:, mi:mi + 1, :])
                if not last:
                    nc.scalar.dma_start(out=st[:, :, off:off + NB], in_=so)

    nc.finalize()
    return nc


def _np_fallback(x, expert_ids, class_anchors, class_expert,
                 W1, b1, g1, be1, W2, b2, g2, be2, W3, b3):
    """Exact dense reference in numpy (used only if capacities overflow or
    the LN params are non-trivial)."""
    def ln(h, g, b):
        mu = h.mean(-1, keepdims=True)
        var = ((h - mu) ** 2).mean(-1, keepdims=True)
        return (h - mu) / np.sqrt(var + EPS) * g + b

    def l2(v):
        n = np.sqrt((v * v).sum(-1, keepdims=True))
        return v / np.maximum(n, 1e-12)

    logits = np.full((B, C), -np.inf, np.float32)
    feats = np.zeros((B, D), np.float32)
    an = l2(class_anchors)
    for e in range(E):
        tok = np.where(expert_ids == e)[0]
        if len(tok) == 0:
            continue
        h = x[tok] @ W1[e] + b1[e]
        h = np.maximum(ln(h, g1[e], be1[e]), 0)
        h = h @ W2[e] + b2[e]
        h = np.maximum(ln(h, g2[e], be2[e]), 0)
        y = h @ W3[e] + b3[e]
        f = l2(y)
        feats[tok] = f
        cls = np.where(class_expert == e)[0]
        if len(cls):
            logits[np.ix_(tok, cls)] = (f @ an[cls].T) / TEMP
    return logits, feats


def _ensure_ntff_hook():
    """run_bass_kernel_spmd(trace=True) needs antenv.axon_hooks, which this
    image lacks; recreate the tiny registry so tracing works if requested."""
    import sys
    import types
    try:
        import antenv  # noqa: F401
        if "antenv.axon_hooks" in sys.modules:
            return
        mod = types.ModuleType("antenv.axon_hooks")
        mod._hook = None
        def _set(h, _m=mod): _m._hook = h
        def _get(_m=mod): return _m._hook
        mod.set_axon_ntff_profile_hook = _set
        mod.get_axon_ntff_profile_hook = _get
        sys.modules["antenv.axon_hooks"] = mod
        from trn_agent_boot.trn_boot import _ntff_profile_via_ctypes
        hook = _ntff_profile_via_ctypes("/opt/axon/libaxon_pjrt.so")
        if hook is not None:
            _set(hook)
    except Exception:
        pass


def kernel(x, expert_ids, class_anchors, class_expert,
           W1, b1, g1, be1, W2, b2, g2, be2, W3, b3):
    from concourse.bass_utils import run_bass_kernel_spmd

    _ensure_ntff_hook()

    x = np.ascontiguousarray(np.asarray(x, np.float32))
    expert_ids = np.asarray(expert_ids).astype(np.int64)
    class_anchors = np.asarray(class_anchors, np.float32)
    class_expert = np.asarray(class_expert).astype(np.int64)
    W1 = np.asarray(W1, np.float32); W2 = np.asarray(W2, np.float32)
    W3 = np.asarray(W3, np.float32)
    b1 = np.asarray(b1, np.float32); b2 = np.asarray(b2, np.float32)
    b3 = np.asarray(b3, np.float32)
    g1 = np.asarray(g1, np.float32); g2 = np.asarray(g2, np.float32)
    be1 = np.asarray(be1, np.float32); be2 = np.asarray(be2, np.float32)

    trivial = (
        not b1.any() and not b2.any() and not b3.any()
        and not be1.any() and not be2.any()
        and (g1 == 1).all() and (g2 == 1).all()
    )
    toks = [np.where(expert_ids == e)[0] for e in range(E)]
    clss = [np.where(class_expert == e)[0] for e in range(E)]
    if (not trivial
            or max(len(t) for t in toks) > CAP
            or max(len(c) for c in clss) > CAPC
            or x.shape != (B, D)):
        return _np_fallback(x, expert_ids, class_anchors, class_expert,
                            W1, b1, g1, be1, W2, b2, g2, be2, W3, b3)

    an = class_anchors / np.maximum(
        np.sqrt((class_anchors ** 2).sum(-1, keepdims=True)), 1e-12
    )

    # fold mean subtraction into W1/W2; fold anchors + 1/TEMP into A
    W1c = W1 - W1.mean(axis=2, keepdims=True)          # [E, D, H]
    W2c = W2 - W2.mean(axis=2, keepdims=True)          # [E, H, H]

    in_maps = []
    for e in range(E):
        tok = toks[e]
        cls = clss[e]
        n = len(tok)
        # x packed as [128, 4, CAP]: xpk[p, k, j] = x[tok[j], k*128+p]
        xpk = np.zeros((128, 4, CAP), np.float32)
        xpk[:, :, :n] = x[tok].T.reshape(4, 128, n).transpose(1, 0, 2)
        wpka = np.ascontiguousarray(
            W1c[e].reshape(4, 128, H).transpose(1, 0, 2).reshape(128, 4 * H))
        wpk2 = np.zeros((128, _WP2_COLS), np.float32)
        wpk2[:, _ONES_O:_ONES_O + 128] = 1.0
        wpk2[:, _W2_O:_W2_O + 2 * H] = (
            W2c[e].reshape(2, 128, H).transpose(1, 0, 2).reshape(128, 2 * H))
        wpk3 = np.zeros((128, _WP3_COLS), np.float32)
        wpk3[:, _W3_O:_W3_O + 2 * D] = (
            W3[e].reshape(2, 128, D).transpose(1, 0, 2).reshape(128, 2 * D))
        A = np.zeros((256, CAPC), np.float32)
        A[:, :len(cls)] = (W3[e] @ an[cls].T) / TEMP
        wpk3[:, _A_O:_A_O + 2 * CAPC] = (
            A.reshape(2, 128, CAPC).transpose(1, 0, 2).reshape(128, 2 * CAPC))
        in_maps.append({"xp": xpk, "wpa": wpka, "wp2": wpk2, "wp3": wpk3})

    if "nc" not in _cache:
        _cache["nc"] = _build_program()
    res = run_bass_kernel_spmd(_cache["nc"], in_maps, list(range(E)))
    _cache["last"] = res

    logits = np.full((B, C), -np.inf, np.float32)
    feats = np.zeros((B, D), np.float32)
    for e in range(E):
        tok = toks[e]
        cls = clss[e]
        n, m = len(tok), len(cls)
        # ft [128, 4, CAP] -> [D, CAP]
        ftr = res.results[e]["ft"].transpose(1, 0, 2).reshape(D, CAP)
        str_ = res.results[e]["st"].transpose(1, 0, 2).reshape(256, CAP)
        feats[tok] = ftr[:, :n].T
        logits[np.ix_(tok, cls)] = str_[:m, :n].T
    return logits, feats


# revision 39
# speedup vs baseline: 1.0433x; 1.0433x over previous
"""Trainium2 Bass kernel for nn_ExpertPool (moe_routing).

Strategy (expert-parallel over 8 cores):
  - Token b only needs its own expert's MLP output, so instead of the dense
    8x dispatch we gather tokens by expert on the host and give expert e's
    tokens to core e (counts ~1024 each, padded to CAP).
  - All activations stay in [feature, token] layout on device, so every
    layer is out = lhsT.T @ rhs with lhsT = weights; no transposes anywhere.
  - setup_inputs uses g=1, be=0, b=0 for every LayerNorm/bias parameter.
    LayerNorm's per-token 1/std commutes through ReLU (positive scale) and
    through the next matmul as a column scale, and both LayerNorm and the
    final l2norm are invariant to per-token positive scales -- so the
    variance/rsqrt path cancels exactly.  Mean subtraction is linear, so it
    folds into the weights host-side (W' = W - rowmean over output dim).
    The device kernel is just: relu(W1'.T x), relu(W2'.T h1), y = W3.T h2,
    feats = y/||y||, sim = (A.T h2)/||y|| with A = (W3 @ anchors_n.T)/TEMP.
  - Class-sim is computed only for the ~125 classes owned by the expert
    (all other logits are exactly -inf by the reference mask).
  - All weights ship in ONE packed [128, 3008] tensor / one DMA; x ships as
    one DMA per token block, prefetched upfront; feats+sim leave as one
    fused DMA each per block (DMA dispatch on the sync sequencer costs
    ~0.6us each, so few+large transfers matter).
"""

import numpy as np

E, B, D, H, C = 8, 8192, 512, 256, 1000
TEMP, EPS = 0.1, 1e-5
CAP = 1152          # per-expert token capacity (seed-0 max: 1040 axon / 1082 cpu)
CAPC = 160          # per-expert class capacity (seed-0 max: 134 axon / 140 cpu)
BLOCKS = [(0, 384), (384, 384), (768, 384)]
MC = [(0, 128), (128, 32)]  # class-chunk (start, size) covering CAPC

# packed weight tensors: wpa = W1, wp2 = ones+W2, wp3 = W3+A
_W1_O = 0              # 4 chunks x 256
_WPA_COLS = _W1_O + 4 * H
_ONES_O = 0
_W2_O = 128            # 2 chunks x 256
_WP2_COLS = _W2_O + 2 * H
_W3_O = 0              # 2 chunks x 512
_A_O = _W3_O + 2 * D   # 2 chunks x 160
_WP3_COLS = _A_O + 2 * CAPC

_cache: dict = {}


def _patch_tile_drain():
    """Walrus in this env rejects >2 sync waits on the tail Drain (CTRL
    encoding limit). Split the waits into standalone wait instructions."""
    import concourse.tile as tile_mod
    from concourse.tile import ScopedClock
    from bass_rust import SemaphoreHandle

    if getattr(tile_mod.TileContext, "_drain_patched", False):
        return

    def _drain_and_barrier(self, tick_clock, wait_clock):
        nc = self.nc
        d1 = nc.sync.drain()
        wait_clock.add_sem_waits(
            d1.ins, ScopedClock({None: tick_clock.global_clock})
        )
        si = d1.ins.sync_info
        waits = list(si.on_wait) if si is not None else []
        if len(waits) > 2:
            d1.ins.sync_info = None
            for w in waits:
                nc.sync.wait_ge(SemaphoreHandle(w.ant_name, w.id), w.wait_value)
            nc.sync.drain()
        nc.all_engine_barrier(sem_only=True)
        assert self.sems is not None
        popped = nc._tile_sem_poison_stack.pop()
        assert popped is self._sem_poison
        nc.clear_and_free_semaphores(list(self.sems.allocated().values()))

    tile_mod.TileContext._drain_and_barrier = _drain_and_barrier
    tile_mod.TileContext._drain_patched = True


def _build_program():
    from contextlib import ExitStack

    import concourse.bacc as bacc
    import concourse.mybir as mybir
    from concourse.tile import TileContext, add_dep_helper

    _patch_tile_drain()

    f32 = mybir.dt.float32
    f32r = mybir.dt.float32r
    AF = mybir.ActivationFunctionType

    nc = bacc.Bacc("TRN2", target_bir_lowering=False, debug=False, num_devices=8)

    xp = nc.dram_tensor("xp", [128, 4, CAP], f32r, kind="ExternalInput")
    wpa = nc.dram_tensor("wpa", [128, _WPA_COLS], f32r, kind="ExternalInput")
    wp2 = nc.dram_tensor("wp2", [128, _WP2_COLS], f32r, kind="ExternalInput")
    wp3 = nc.dram_tensor("wp3", [128, _WP3_COLS], f32r, kind="ExternalInput")
    ft = nc.dram_tensor("ft", [128, 4, CAP], f32, kind="ExternalOutput")
    st = nc.dram_tensor("st", [128, 2, CAP], f32, kind="ExternalOutput")

    with TileContext(nc) as tc:
        with ExitStack() as ctx:
            wpool = ctx.enter_context(tc.tile_pool(name="wpool", bufs=1))
            xpool = ctx.enter_context(tc.tile_pool(name="xpool", bufs=1))
            hpool = ctx.enter_context(tc.tile_pool(name="hpool", bufs=2))
            qpool = ctx.enter_context(tc.tile_pool(name="qpool", bufs=2))
            opool = ctx.enter_context(tc.tile_pool(name="opool", bufs=2))
            psA = ctx.enter_context(tc.tile_pool(name="psA", bufs=2, space="PSUM"))
            psY = ctx.enter_context(tc.tile_pool(name="psY", bufs=4, space="PSUM"))
            psS = ctx.enter_context(tc.tile_pool(name="psS", bufs=2, space="PSUM"))

            wpa_sb = wpool.tile([128, _WPA_COLS], f32r, tag="wpa")
            d_wpa = nc.scalar.dma_start(out=wpa_sb, in_=wpa[:, :])
            wp2_sb = wpool.tile([128, _WP2_COLS], f32r, tag="wp2")
            d_wp2 = nc.scalar.dma_start(out=wp2_sb, in_=wp2[:, :])
            wp3_sb = wpool.tile([128, _WP3_COLS], f32r, tag="wp3")
            d_wp3 = nc.scalar.dma_start(out=wp3_sb, in_=wp3[:, :])
            ones_r = wp2_sb[:, _ONES_O:_ONES_O + 128]
            w1_sb = [wpa_sb[:, _W1_O + k * H:_W1_O + (k + 1) * H]
                     for k in range(4)]
            w2_sb = [wp2_sb[:, _W2_O + k * H:_W2_O + (k + 1) * H]
                     for k in range(2)]
            w3_sb = [wp3_sb[:, _W3_O + k * D:_W3_O + (k + 1) * D]
                     for k in range(2)]
            a_sb = [wp3_sb[:, _A_O + k * CAPC:_A_O + (k + 1) * CAPC]
                    for k in range(2)]
            eps_sb = wpool.tile([128, 1], f32, tag="eps")
            nc.vector.memset(eps_sb, 1e-30)
            scr_sb = wpool.tile([128, 1], f32, tag="scr")
            nc.scalar.activation(out=scr_sb, in_=eps_sb, func=AF.Sqrt)

            # PE warm-up: keep the HAM activity window busy while input DMAs
            # are in flight so real matmuls start at 2.4 GHz
            bf16 = mybir.dt.bfloat16
            warm = wpool.tile([128, 384], bf16, tag="warm")
            nc.vector.memset(warm, 0.0)
            wps = psS.tile([128, 256], f32, tag="sc")
            for i in range(20):
                nc.tensor.matmul(wps, lhsT=warm[:, 0:128],
                                 rhs=warm[:, 128:384],
                                 start=(i == 0), stop=(i == 19))

            # prefetch every token block upfront (one DMA per block)
            # x0 from the (otherwise idle) scalar queue so it overlaps wpa;
            # x1/x2 from sync, chained so they don't steal bandwidth from the
            # critical wpa+x0 transfers. The chain waits run on the sync
            # sequencer, which has nothing else to do until block-0 outputs.
            # x blocks all on the scalar HWDGE queue: its FIFO order gives
            # x0 -> x1 -> x2 while weights stream on the sync queue in
            # parallel (each engine has one HW queue; ~290 GB/s each)
            xts = []
            for bi, (off, NB) in enumerate(BLOCKS):
                t = xpool.tile([128, 4, NB], f32r, tag=f"xt{bi}")
                d = nc.sync.dma_start(out=t, in_=xp[:, :, off:off + NB])
                xts.append(t)

            for bi, (off, NB) in enumerate(BLOCKS):
                xt = xts[bi]

                # -------- layer 1: h1 = relu(W1'.T x) --------
                h1 = []
                for hc in range(2):
                    ps = psA.tile([128, NB], f32, tag="zps")
                    for k in range(4):
                        nc.tensor.matmul(
                            ps,
                            lhsT=w1_sb[k][:, hc * 128:(hc + 1) * 128],
                            rhs=xt[:, k, :],
                            start=(k == 0), stop=(k == 3),
                        )
                    h = hpool.tile([128, NB], f32r, tag=f"h1_{hc}")
                    nc.scalar.activation(out=h, in_=ps, func=AF.Relu)
                    h1.append(h)

                # -------- layer 2: h2 = relu(W2'.T h1) --------
                h2 = []
                for hc in range(2):
                    ps = psA.tile([128, NB], f32, tag="zps")
                    for k in range(2):
                        nc.tensor.matmul(
                            ps,
                            lhsT=w2_sb[k][:, hc * 128:(hc + 1) * 128],
                            rhs=h1[k][:, :],
                            start=(k == 0), stop=(k == 1),
                        )
                    h = hpool.tile([128, NB], f32r, tag=f"h2_{hc}")
                    nc.scalar.activation(out=h, in_=ps, func=AF.Relu)
                    h2.append(h)

                # -------- layer 3: y = W3.T h2 (stays in PSUM) --------
                y_ps = []
                for dc in range(4):
                    ps = psY.tile([128, NB], f32, tag="yps")
                    for k in range(2):
                        nc.tensor.matmul(
                            ps,
                            lhsT=w3_sb[k][:, dc * 128:(dc + 1) * 128],
                            rhs=h2[k][:, :],
                            start=(k == 0), stop=(k == 1),
                        )
                    y_ps.append(ps)

                # -------- ||y||^2 across partitions via ones-matmul --------
                ysq = []
                for dc in range(4):
                    q = qpool.tile([128, NB], f32r, tag=f"ysq{dc}")
                    nc.scalar.activation(out=q, in_=y_ps[dc], func=AF.Square)
                    ysq.append(q)
                ss_ps = psS.tile([128, NB], f32, tag="sc")
                for dc in range(4):
                    nc.tensor.matmul(
                        ss_ps, lhsT=ones_r, rhs=ysq[dc][:, :],
                        start=(dc == 0), stop=(dc == 3),
                    )
                std = qpool.tile([128, NB], f32, tag="std")
                nc.scalar.activation(out=std, in_=ss_ps, func=AF.Sqrt, bias=eps_sb)
                s_b = qpool.tile([128, NB], f32, tag="s_b")
                nc.vector.reciprocal_approx_fast(out=s_b, in_=std)

                # -------- feats = y * s --------
                last = (bi == len(BLOCKS) - 1)
                fo = opool.tile([128, 4, NB], f32, tag="fo")
                for dc in range(4):
                    nc.vector.tensor_mul(fo[:, dc, :], y_ps[dc], s_b)
                    if dc % 2 == 1:
                        nc.sync.dma_start(
                            out=ft[:, dc - 1:dc + 1, off:off + NB],
                            in_=fo[:, dc - 1:dc + 1, :])

                # -------- sim = (A.T h2) * s (A has 1/TEMP baked in) --------
                so = opool.tile([128, 2, NB], f32, tag="so")
                for mi, (mstart, msize) in enumerate(MC):
                    ps = psS.tile([128, NB], f32, tag="sc")
                    for k in range(2):
                        nc.tensor.matmul(
                            ps[0:msize, :],
                            lhsT=a_sb[k][:, mstart:mstart + msize],
                            rhs=h2[k][:, :],
                            start=(k == 0), stop=(k == 1),
                        )
                    nc.vector.tensor_mul(
                        so[0:msize, mi, :], ps[0:msize, :], s_b[0:msize, :]
                    )
                    if last:
                        nc.scalar.dma_start(
                            out=st[:, mi:mi + 1, off:off + NB],
                            in_=so[:, mi:mi + 1, :])
                if not last:
                    nc.scalar.dma_start(out=st[:, :, off:off + NB], in_=so)

    nc.finalize()
    return nc


def _np_fallback(x, expert_ids, class_anchors, class_expert,
                 W1, b1, g1, be1, W2, b2, g2, be2, W3, b3):
    """Exact dense reference in numpy (used only if capacities overflow or
    the LN params are non-trivial)."""
    def ln(h, g, b):
        mu = h.mean(-1, keepdims=True)
        var = ((h - mu) ** 2).mean(-1, keepdims=True)
        return (h - mu) / np.sqrt(var + EPS) * g + b

    def l2(v):
        n = np.sqrt((v * v).sum(-1, keepdims=True))
        return v / np.maximum(n, 1e-12)

    logits = np.full((B, C), -np.inf, np.float32)
    feats = np.zeros((B, D), np.float32)
    an = l2(class_anchors)
    for e in range(E):
        tok = np.where(expert_ids == e)[0]
        if len(tok) == 0:
            continue
        h = x[tok] @ W1[e] + b1[e]
        h = np.maximum(ln(h, g1[e], be1[e]), 0)
        h = h @ W2[e] + b2[e]
        h = np.maximum(ln(h, g2[e], be2[e]), 0)
        y = h @ W3[e] + b3[e]
        f = l2(y)
        feats[tok] = f
        cls = np.where(class_expert == e)[0]
        if len(cls):
            logits[np.ix_(tok, cls)] = (f @ an[cls].T) / TEMP
    return logits, feats


def _ensure_ntff_hook():
    """run_bass_kernel_spmd(trace=True) needs antenv.axon_hooks, which this
    image lacks; recreate the tiny registry so tracing works if requested."""
    import sys
    import types
    try:
        import antenv  # noqa: F401
        if "antenv.axon_hooks" in sys.modules:
            return
        mod = types.ModuleType("antenv.axon_hooks")
        mod._hook = None
        def _set(h, _m=mod): _m._hook = h
        def _get(_m=mod): return _m._hook
        mod.set_axon_ntff_profile_hook = _set
        mod.get_axon_ntff_profile_hook = _get
        sys.modules["antenv.axon_hooks"] = mod
        from trn_agent_boot.trn_boot import _ntff_profile_via_ctypes
        hook = _ntff_profile_via_ctypes("/opt/axon/libaxon_pjrt.so")
        if hook is not None:
            _set(hook)
    except Exception:
        pass


def kernel(x, expert_ids, class_anchors, class_expert,
           W1, b1, g1, be1, W2, b2, g2, be2, W3, b3):
    from concourse.bass_utils import run_bass_kernel_spmd

    _ensure_ntff_hook()

    x = np.ascontiguousarray(np.asarray(x, np.float32))
    expert_ids = np.asarray(expert_ids).astype(np.int64)
    class_anchors = np.asarray(class_anchors, np.float32)
    class_expert = np.asarray(class_expert).astype(np.int64)
    W1 = np.asarray(W1, np.float32); W2 = np.asarray(W2, np.float32)
    W3 = np.asarray(W3, np.float32)
    b1 = np.asarray(b1, np.float32); b2 = np.asarray(b2, np.float32)
    b3 = np.asarray(b3, np.float32)
    g1 = np.asarray(g1, np.float32); g2 = np.asarray(g2, np.float32)
    be1 = np.asarray(be1, np.float32); be2 = np.asarray(be2, np.float32)

    trivial = (
        not b1.any() and not b2.any() and not b3.any()
        and not be1.any() and not be2.any()
        and (g1 == 1).all() and (g2 == 1).all()
    )
    toks = [np.where(expert_ids == e)[0] for e in range(E)]
    clss = [np.where(class_expert == e)[0] for e in range(E)]
    if (not trivial
            or max(len(t) for t in toks) > CAP
            or max(len(c) for c in clss) > CAPC
            or x.shape != (B, D)):
        return _np_fallback(x, expert_ids, class_anchors, class_expert,
                            W1, b1, g1, be1, W2, b2, g2, be2, W3, b3)

    an = class_anchors / np.maximum(
        np.sqrt((class_anchors ** 2).sum(-1, keepdims=True)), 1e-12
    )

    # fold mean subtraction into W1/W2; fold anchors + 1/TEMP into A
    W1c = W1 - W1.mean(axis=2, keepdims=True)          # [E, D, H]
    W2c = W2 - W2.mean(axis=2, keepdims=True)          # [E, H, H]

    in_maps = []
    for e in range(E):
        tok = toks[e]
        cls = clss[e]
        n = len(tok)
        # x packed as [128, 4, CAP]: xpk[p, k, j] = x[tok[j], k*128+p]
        xpk = np.zeros((128, 4, CAP), np.float32)
        xpk[:, :, :n] = x[tok].T.reshape(4, 128, n).transpose(1, 0, 2)
        wpka = np.ascontiguousarray(
            W1c[e].reshape(4, 128, H).transpose(1, 0, 2).reshape(128, 4 * H))
        wpk2 = np.zeros((128, _WP2_COLS), np.float32)
        wpk2[:, _ONES_O:_ONES_O + 128] = 1.0
        wpk2[:, _W2_O:_W2_O + 2 * H] = (
            W2c[e].reshape(2, 128, H).transpose(1, 0, 2).reshape(128, 2 * H))
        wpk3 = np.zeros((128, _WP3_COLS), np.float32)
        wpk3[:, _W3_O:_W3_O + 2 * D] = (
            W3[e].reshape(2, 128, D).transpose(1, 0, 2).reshape(128, 2 * D))
        A = np.zeros((256, CAPC), np.float32)
        A[:, :len(cls)] = (W3[e] @ an[cls].T) / TEMP
        wpk3[:, _A_O:_A_O + 2 * CAPC] = (
            A.reshape(2, 128, CAPC).transpose(1, 0, 2).reshape(128, 2 * CAPC))
        in_maps.append({"xp": xpk, "wpa": wpka, "wp2": wpk2, "wp3": wpk3})

    if "nc" not in _cache:
        _cache["nc"] = _build_program()
    res = run_bass_kernel_spmd(_cache["nc"], in_maps, list(range(E)))
    _cache["last"] = res

    logits = np.full((B, C), -np.inf, np.float32)
    feats = np.zeros((B, D), np.float32)
    for e in range(E):
        tok = toks[e]
        cls = clss[e]
        n, m = len(tok), len(cls)
        # ft [128, 4, CAP] -> [D, CAP]
        ftr = res.results[e]["ft"].transpose(1, 0, 2).reshape(D, CAP)
        str_ = res.results[e]["st"].transpose(1, 0, 2).reshape(256, CAP)
        feats[tok] = ftr[:, :n].T
        logits[np.ix_(tok, cls)] = str_[:m, :n].T
    return logits, feats
